# revision 24
# baseline (speedup 1.0000x reference)
"""Trainium2 Bass kernel for nn_BlockDiagonalLinearAlignment.

Math: y = x @ A, where A is a 128x128 block-diagonal matrix assembled from
dense / diagonal / low-rank 16x16 blocks, followed by row-wise L2
normalization: out = y / (||y||_2 + 1e-8).

Strategy (pure data parallel over the batch axis, 8 cores):
  - per core: 32768 rows of x [*, 128] fp32.
  - chunked processing: CHUNK rows per DMA (contiguous, 128-partition layout,
    partition p holds rows [16p, 16p+16) of the chunk).
  - per 128-row tile: PE transpose (matmul vs identity) -> xT in PSUM,
    DVE copy PSUM->SBUF, PE matmul (lhsT=xT, rhs=A) -> y batch-major in PSUM,
    ACT square+accumulate -> ||y||^2, ACT sqrt, DVE reciprocal,
    DVE tensor_tensor multiply with a stride-0 broadcast of 1/||y||.
"""

import contextlib
import functools
import sys

for _p in ("/opt/trn_rl_repo",):
    if _p not in sys.path:
        sys.path.append(_p)

import numpy as np

import concourse.bacc as bacc
import concourse.bass as bass
import concourse.tile as tile
from concourse import bass_utils, mybir

B = 262144
D = 128
BS = 16
K = 8
N_CORES = 8
ROWS_PER_CORE = B // N_CORES  # 32768

DENSE = (0, 3, 6)
DIAG = (1, 4, 7)
LR = (2, 5)

F32 = mybir.dt.float32

CHUNK_ROWS = 4096  # rows per DMA chunk (per core)
P = 128

# implementation variants (bisect/perf knobs)
SQUARE_MODE = "act512"      # "act_accum" | "dve" | "act512"
SCALE_MODE = "tt_bcast"     # "tt_bcast" | "act_copy" | "ts"
XT_COPY_ENGINE = "scalar"   # "vector" | "scalar"
F32R = False                # float32r: faster matmul but rel err ~1.5e-4 (HW)
GROUP_TILES = 8             # 128-row tiles per PSUM group (4 -> 1 bank, 8 -> 2)
BUFS = dict(inpool=3, outpool=3, xtpool=4, sqpool=3, smalls=8, psA=2, psB=2)


def _assemble_A(W_dense, s_diag, U, V):
    """Full 128x128 block-diagonal transform, y = x @ A."""
    A = np.zeros((D, D), dtype=np.float32)
    for i, k in enumerate(DENSE):
        A[k * BS:(k + 1) * BS, k * BS:(k + 1) * BS] = W_dense[i].T
    for i, k in enumerate(DIAG):
        A[k * BS:(k + 1) * BS, k * BS:(k + 1) * BS] = np.diag(s_diag[i])
    for i, k in enumerate(LR):
        A[k * BS:(k + 1) * BS, k * BS:(k + 1) * BS] = V[i] @ U[i].T
    return A


def _kernel_body(ctx, tc, out_ap, x_ap, amat_ap, ident_ap, rows, chunk_rows):
    nc = tc.nc
    rpp = chunk_rows // P          # rows per partition per chunk
    nchunks = rows // chunk_rows
    gt = GROUP_TILES
    ngroups = rpp // gt            # tiles per PSUM group
    assert rpp % gt == 0 and rows % chunk_rows == 0

    xv = x_ap.rearrange("(c p r) f -> c p r f", c=nchunks, p=P)
    ov = out_ap.rearrange("(c p r) f -> c p r f", c=nchunks, p=P)

    MMDT = mybir.dt.float32r if F32R else F32
    AW = 2 if F32R else 1       # A replicated AW times along N (f32r: N>=256)

    consts = ctx.enter_context(tc.tile_pool(name="consts", bufs=1))
    ident = consts.tile([P, P], MMDT)
    nc.sync.dma_start(out=ident, in_=ident_ap)
    amat = consts.tile([P, AW, P], MMDT)
    for w in range(AW):
        nc.sync.dma_start(out=amat[:, w, :], in_=amat_ap)

    inpool = ctx.enter_context(tc.tile_pool(name="inpool", bufs=BUFS["inpool"]))
    outpool = ctx.enter_context(tc.tile_pool(name="outpool", bufs=BUFS["outpool"]))
    xtpool = ctx.enter_context(tc.tile_pool(name="xtpool", bufs=BUFS["xtpool"]))
    sqpool = ctx.enter_context(tc.tile_pool(name="sqpool", bufs=BUFS["sqpool"]))
    smalls = ctx.enter_context(tc.tile_pool(name="smalls", bufs=BUFS["smalls"]))
    psA = ctx.enter_context(tc.tile_pool(name="psA", bufs=BUFS["psA"], space="PSUM"))
    psB = ctx.enter_context(tc.tile_pool(name="psB", bufs=BUFS["psB"], space="PSUM"))

    for c in range(nchunks):
        in_sb = inpool.tile([P, rpp, D], MMDT)
        nc.sync.dma_start(out=in_sb, in_=xv[c])
        out_sb = outpool.tile([P, rpp, D], F32)

        for g in range(ngroups):
            xT_ps = psA.tile([P, gt, D], MMDT)  # transpose out dtype == in dtype
            for j in range(gt):
                nc.tensor.transpose(xT_ps[:, j], in_sb[:, g * gt + j, :], ident)
            xT_sb = xtpool.tile([P, gt, D], MMDT)
            if XT_COPY_ENGINE == "vector":
                nc.vector.tensor_copy(xT_sb, xT_ps)
            else:
                nc.scalar.copy(xT_sb, xT_ps)

            y_ps = psB.tile([P, gt, AW * D], F32)
            for j in range(gt):
                nc.tensor.matmul(
                    y_ps[:, j], lhsT=xT_sb[:, j], rhs=amat,
                    start=True, stop=True,
                )

            yv = y_ps[:, :, 0:D] if AW > 1 else y_ps

            n2 = smalls.tile([P, gt], F32)
            if SQUARE_MODE == "act_accum":
                sq = sqpool.tile([P, gt, D], F32)
                for j in range(gt):
                    nc.scalar.activation(
                        sq[:, j], yv[:, j],
                        mybir.ActivationFunctionType.Square,
                        accum_out=n2[:, j:j + 1],
                    )
            elif SQUARE_MODE == "act512":
                sq = sqpool.tile([P, gt, D], F32)
                nc.scalar.activation(
                    sq, yv, mybir.ActivationFunctionType.Square,
                )
                nc.vector.tensor_reduce(
                    n2, sq, axis=mybir.AxisListType.X, op=mybir.AluOpType.add,
                )
            else:  # "dve"
                sq = sqpool.tile([P, gt, D], F32)
                nc.vector.tensor_mul(sq, yv, yv)
                nc.vector.tensor_reduce(
                    n2, sq, axis=mybir.AxisListType.X, op=mybir.AluOpType.add,
                )
            nrm = smalls.tile([P, gt], F32)
            nc.scalar.sqrt(nrm, n2)
            rnorm = smalls.tile([P, gt], F32)
            nc.vector.reciprocal(rnorm, nrm)

            if SCALE_MODE == "tt_bcast":
                nc.vector.tensor_mul(
                    out_sb[:, g * gt:(g + 1) * gt, :],
                    yv,
                    rnorm.broadcast_to([P, gt, D]),
                )
            elif SCALE_MODE == "act_copy":
                for j in range(gt):
                    nc.scalar.activation(
                        out_sb[:, g * gt + j, :], yv[:, j],
                        mybir.ActivationFunctionType.Copy,
                        scale=rnorm[:, j:j + 1],
                    )
            else:  # "ts"
                for j in range(gt):
                    nc.vector.tensor_scalar_mul(
                        out_sb[:, g * gt + j, :], yv[:, j],
                        rnorm[:, j:j + 1],
                    )

        nc.sync.dma_start(out=ov[c], in_=out_sb)


@functools.lru_cache(maxsize=4)
def _build(rows, chunk_rows):
    nc = bacc.Bacc(
        "TRN2",
        target_bir_lowering=False,
        debug=False,
        num_devices=1,
    )
    mmdt = mybir.dt.float32r if F32R else F32
    x_t = nc.dram_tensor("x", [rows, D], mmdt, kind="ExternalInput").ap()
    a_t = nc.dram_tensor("amat", [D, D], mmdt, kind="ExternalInput").ap()
    i_t = nc.dram_tensor("ident", [D, D], mmdt, kind="ExternalInput").ap()
    o_t = nc.dram_tensor("out", [rows, D], F32, kind="ExternalOutput").ap()
    with tile.TileContext(nc) as tc, contextlib.ExitStack() as ctx:
        _kernel_body(ctx, tc, o_t, x_t, a_t, i_t, rows, chunk_rows)
    nc.compile()
    return nc


def _run(x, A, trace=False, trace_cores=None):
    nc = _build(ROWS_PER_CORE, CHUNK_ROWS)
    xs = np.ascontiguousarray(x.reshape(N_CORES, ROWS_PER_CORE, D))
    ident = np.eye(D, dtype=np.float32)
    in_maps = [{"x": xs[i], "amat": A, "ident": ident} for i in range(N_CORES)]
    res = bass_utils.run_bass_kernel_spmd(
        nc, in_maps, core_ids=list(range(N_CORES)),
        trace=trace, trace_cores=trace_cores,
    )
    out = np.concatenate([r["out"] for r in res.results], axis=0)
    return out, res


def kernel(x, W_dense, s_diag, U, V):
    A = _assemble_A(
        np.asarray(W_dense, dtype=np.float32),
        np.asarray(s_diag, dtype=np.float32),
        np.asarray(U, dtype=np.float32),
        np.asarray(V, dtype=np.float32),
    )
    out, _ = _run(np.asarray(x, dtype=np.float32), A)
    return out


# revision 25
# speedup vs baseline: 1.3063x; 1.3063x over previous
"""Trainium2 Bass kernel for nn_BlockDiagonalLinearAlignment.

Math: y = x @ A, where A is a 128x128 block-diagonal matrix assembled from
dense / diagonal / low-rank 16x16 blocks, followed by row-wise L2
normalization: out = y / (||y||_2 + 1e-8).

Strategy (pure data parallel over the batch axis, 8 cores):
  - per core: 32768 rows of x [*, 128] fp32.
  - chunked processing: CHUNK rows per DMA (contiguous, 128-partition layout,
    partition p holds rows [16p, 16p+16) of the chunk).
  - per 128-row tile: PE transpose (matmul vs identity) -> xT in PSUM,
    DVE copy PSUM->SBUF, PE matmul (lhsT=xT, rhs=A) -> y batch-major in PSUM,
    ACT square+accumulate -> ||y||^2, ACT sqrt, DVE reciprocal,
    DVE tensor_tensor multiply with a stride-0 broadcast of 1/||y||.
"""

import contextlib
import functools
import sys

for _p in ("/opt/trn_rl_repo",):
    if _p not in sys.path:
        sys.path.append(_p)

import numpy as np

import concourse.bacc as bacc
import concourse.bass as bass
import concourse.tile as tile
from concourse import bass_utils, mybir

B = 262144
D = 128
BS = 16
K = 8
N_CORES = 8
ROWS_PER_CORE = B // N_CORES  # 32768

DENSE = (0, 3, 6)
DIAG = (1, 4, 7)
LR = (2, 5)

F32 = mybir.dt.float32

CHUNK_ROWS = 4096  # rows per DMA chunk (per core)
P = 128

# implementation variants (bisect/perf knobs)
SQUARE_MODE = "act512"      # "act_accum" | "dve" | "act512"
SCALE_MODE = "tt_bcast"     # "tt_bcast" | "act_copy" | "ts"
XT_COPY_ENGINE = "scalar"   # "vector" | "scalar"
F32R = False                # float32r: faster matmul but rel err ~1.5e-4 (HW)
GROUP_TILES = 4             # 128-row tiles per PSUM group (4 -> 1 bank, 8 -> 2)
BUFS = dict(inpool=3, outpool=3, xtpool=6, sqpool=4, smalls=8, psA=4, psB=4)


def _assemble_A(W_dense, s_diag, U, V):
    """Full 128x128 block-diagonal transform, y = x @ A."""
    A = np.zeros((D, D), dtype=np.float32)
    for i, k in enumerate(DENSE):
        A[k * BS:(k + 1) * BS, k * BS:(k + 1) * BS] = W_dense[i].T
    for i, k in enumerate(DIAG):
        A[k * BS:(k + 1) * BS, k * BS:(k + 1) * BS] = np.diag(s_diag[i])
    for i, k in enumerate(LR):
        A[k * BS:(k + 1) * BS, k * BS:(k + 1) * BS] = V[i] @ U[i].T
    return A


def _kernel_body(ctx, tc, out_ap, x_ap, amat_ap, ident_ap, rows, chunk_rows):
    nc = tc.nc
    rpp = chunk_rows // P          # rows per partition per chunk
    nchunks = rows // chunk_rows
    gt = GROUP_TILES
    ngroups = rpp // gt            # tiles per PSUM group
    assert rpp % gt == 0 and rows % chunk_rows == 0

    xv = x_ap.rearrange("(c p r) f -> c p r f", c=nchunks, p=P)
    ov = out_ap.rearrange("(c p r) f -> c p r f", c=nchunks, p=P)

    MMDT = mybir.dt.float32r if F32R else F32
    AW = 2 if F32R else 1       # A replicated AW times along N (f32r: N>=256)

    consts = ctx.enter_context(tc.tile_pool(name="consts", bufs=1))
    ident = consts.tile([P, P], MMDT)
    nc.sync.dma_start(out=ident, in_=ident_ap)
    amat = consts.tile([P, AW, P], MMDT)
    for w in range(AW):
        nc.sync.dma_start(out=amat[:, w, :], in_=amat_ap)

    inpool = ctx.enter_context(tc.tile_pool(name="inpool", bufs=BUFS["inpool"]))
    outpool = ctx.enter_context(tc.tile_pool(name="outpool", bufs=BUFS["outpool"]))
    xtpool = ctx.enter_context(tc.tile_pool(name="xtpool", bufs=BUFS["xtpool"]))
    sqpool = ctx.enter_context(tc.tile_pool(name="sqpool", bufs=BUFS["sqpool"]))
    smalls = ctx.enter_context(tc.tile_pool(name="smalls", bufs=BUFS["smalls"]))
    psA = ctx.enter_context(tc.tile_pool(name="psA", bufs=BUFS["psA"], space="PSUM"))
    psB = ctx.enter_context(tc.tile_pool(name="psB", bufs=BUFS["psB"], space="PSUM"))

    for c in range(nchunks):
        in_sb = inpool.tile([P, rpp, D], MMDT)
        nc.sync.dma_start(out=in_sb, in_=xv[c])
        out_sb = outpool.tile([P, rpp, D], F32)

        for g in range(ngroups):
            xT_ps = psA.tile([P, gt, D], MMDT)  # transpose out dtype == in dtype
            for j in range(gt):
                nc.tensor.transpose(xT_ps[:, j], in_sb[:, g * gt + j, :], ident)
            xT_sb = xtpool.tile([P, gt, D], MMDT)
            if XT_COPY_ENGINE == "vector":
                nc.vector.tensor_copy(xT_sb, xT_ps)
            else:
                nc.scalar.copy(xT_sb, xT_ps)

            y_ps = psB.tile([P, gt, AW * D], F32)
            for j in range(gt):
                nc.tensor.matmul(
                    y_ps[:, j], lhsT=xT_sb[:, j], rhs=amat,
                    start=True, stop=True,
                )

            yv = y_ps[:, :, 0:D] if AW > 1 else y_ps

            n2 = smalls.tile([P, gt], F32)
            if SQUARE_MODE == "act_accum":
                sq = sqpool.tile([P, gt, D], F32)
                for j in range(gt):
                    nc.scalar.activation(
                        sq[:, j], yv[:, j],
                        mybir.ActivationFunctionType.Square,
                        accum_out=n2[:, j:j + 1],
                    )
            elif SQUARE_MODE == "act512":
                sq = sqpool.tile([P, gt, D], F32)
                nc.scalar.activation(
                    sq, yv, mybir.ActivationFunctionType.Square,
                )
                nc.vector.tensor_reduce(
                    n2, sq, axis=mybir.AxisListType.X, op=mybir.AluOpType.add,
                )
            else:  # "dve"
                sq = sqpool.tile([P, gt, D], F32)
                nc.vector.tensor_mul(sq, yv, yv)
                nc.vector.tensor_reduce(
                    n2, sq, axis=mybir.AxisListType.X, op=mybir.AluOpType.add,
                )
            nrm = smalls.tile([P, gt], F32)
            nc.scalar.sqrt(nrm, n2)
            rnorm = smalls.tile([P, gt], F32)
            nc.vector.reciprocal(rnorm, nrm)

            if SCALE_MODE == "tt_bcast":
                nc.vector.tensor_mul(
                    out_sb[:, g * gt:(g + 1) * gt, :],
                    yv,
                    rnorm.broadcast_to([P, gt, D]),
                )
            elif SCALE_MODE == "act_copy":
                for j in range(gt):
                    nc.scalar.activation(
                        out_sb[:, g * gt + j, :], yv[:, j],
                        mybir.ActivationFunctionType.Copy,
                        scale=rnorm[:, j:j + 1],
                    )
            else:  # "ts"
                for j in range(gt):
                    nc.vector.tensor_scalar_mul(
                        out_sb[:, g * gt + j, :], yv[:, j],
                        rnorm[:, j:j + 1],
                    )

        nc.sync.dma_start(out=ov[c], in_=out_sb)


@functools.lru_cache(maxsize=4)
def _build(rows, chunk_rows):
    nc = bacc.Bacc(
        "TRN2",
        target_bir_lowering=False,
        debug=False,
        num_devices=1,
    )
    mmdt = mybir.dt.float32r if F32R else F32
    x_t = nc.dram_tensor("x", [rows, D], mmdt, kind="ExternalInput").ap()
    a_t = nc.dram_tensor("amat", [D, D], mmdt, kind="ExternalInput").ap()
    i_t = nc.dram_tensor("ident", [D, D], mmdt, kind="ExternalInput").ap()
    o_t = nc.dram_tensor("out", [rows, D], F32, kind="ExternalOutput").ap()
    with tile.TileContext(nc) as tc, contextlib.ExitStack() as ctx:
        _kernel_body(ctx, tc, o_t, x_t, a_t, i_t, rows, chunk_rows)
    nc.compile()
    return nc


def _run(x, A, trace=False, trace_cores=None):
    nc = _build(ROWS_PER_CORE, CHUNK_ROWS)
    xs = np.ascontiguousarray(x.reshape(N_CORES, ROWS_PER_CORE, D))
    ident = np.eye(D, dtype=np.float32)
    in_maps = [{"x": xs[i], "amat": A, "ident": ident} for i in range(N_CORES)]
    res = bass_utils.run_bass_kernel_spmd(
        nc, in_maps, core_ids=list(range(N_CORES)),
        trace=trace, trace_cores=trace_cores,
    )
    out = np.concatenate([r["out"] for r in res.results], axis=0)
    return out, res


def kernel(x, W_dense, s_diag, U, V):
    A = _assemble_A(
        np.asarray(W_dense, dtype=np.float32),
        np.asarray(s_diag, dtype=np.float32),
        np.asarray(U, dtype=np.float32),
        np.asarray(V, dtype=np.float32),
    )
    out, _ = _run(np.asarray(x, dtype=np.float32), A)
    return out
